# revision 13
# baseline (speedup 1.0000x reference)
"""Bass/Trainium2 kernel for DecodeMultiHeadAttention (16 heads, B=2, T=1024, C=1024).

Sharding: tensor-parallel over heads -- 2 heads per core x 8 cores.

Math notes (vs the jax reference):
  * The ALiBi bias is base**clip(j-i,0) which is exactly 1.0 on every causal
    (unmasked) position, and softmax is shift-invariant, so it drops out.
  * Scores are tiny (std ~0.1) so softmax needs no max-subtraction:
    wei = exp(s*scale) / sum(exp(s*scale)) over the causal extent.

Per core (2 local heads h in {0,1}, batch b in {0,1}):
  * qT,kT = [d,t]-layout projections (PE contracts embedding dim c, using a
    host-transposed xT input so all DMA is contiguous).
  * v in natural [t,d] layout with a ones-column appended, so a single PE
    matmul per pair produces both (p @ v)^T and the softmax denominator.
  * Scores are computed transposed, sT[s,t] = k_s . q_t, so the p @ v matmul
    needs no on-chip transpose. wei is written transposed ([s,t]) and the
    host swaps the last two axes while unsharding.
  * Only the causal (lower-triangle in [t,s] == upper in [s,t]) block rows are
    computed and written; the rest of the wei output buffer stays zero
    (outputs are zero-initialized).
  * Output projection: each core computes a partial product over its 128 head
    dims (+ bias/8); the host sums the 8 partials.
"""

import numpy as np

import concourse.bacc as bacc
import concourse.tile as tile
from concourse import mybir
from concourse import bass_utils

N_CORES = 8
B, T, C = 2, 1024, 1024
H, D = 16, 64          # total heads, head dim
HL = H // N_CORES      # heads per core (2)
NT = B * T             # 2048 rows total
P = 128
KC = C // P            # 8 contraction tiles over embedding dim
TT = NT // P           # 16 row tiles
SM = T // P            # 8 s-tiles per (head, batch) pair
SCALE = float(C) ** -0.5
F32 = mybir.dt.float32
BF16 = mybir.dt.bfloat16

_CACHED = {}


def _body(tc, xT, wq, wk, wv, projT, bias16, weiT, partial, ctx, pfx=""):
    nc = tc.nc
    Exp = mybir.ActivationFunctionType.Exp

    const = ctx.enter_context(tc.tile_pool(name=pfx + "const", bufs=1))
    stage = ctx.enter_context(tc.tile_pool(name=pfx + "stage", bufs=2))
    ppool = ctx.enter_context(tc.tile_pool(name=pfx + "ppool", bufs=3))
    wpool = ctx.enter_context(tc.tile_pool(name=pfx + "wpool", bufs=4))
    rpool = ctx.enter_context(tc.tile_pool(name=pfx + "rpool", bufs=2))
    opool = ctx.enter_context(tc.tile_pool(name=pfx + "opool", bufs=4))
    psum = ctx.enter_context(
        tc.tile_pool(name=pfx + "psum", bufs=3, space="PSUM"))
    psum_oa = ctx.enter_context(
        tc.tile_pool(name=pfx + "psum_oa", bufs=2, space="PSUM"))

    # ---- persistent SBUF tensors ----
    xbf = const.tile([P, KC, NT], BF16)       # x^T, bf16: [c-part, c-tile, t]
    wbf = const.tile([P, 3, KC, P], BF16)     # q/k/v weights: [c-part, proj, c-tile, d]
    qTb = const.tile([P, NT], BF16)           # q^T: [2 heads x 64 d, t]
    kTb = const.tile([P, NT], BF16)
    v_sb = const.tile([P, TT, 2 * (D + 1)], BF16)  # v natural + ones cols
    # projT halves with a trailing bias/16 row: proj matmul K=65 adds bias
    pj0 = const.tile([D + 1, C], BF16)
    pj1 = const.tile([D + 1, C], BF16)
    # head-out^T with a trailing ones row (multiplies the bias row)
    hoT0 = const.tile([D + 1, NT], BF16)
    hoT1 = const.tile([D + 1, NT], BF16)
    nc.vector.memset(hoT0[D:D + 1, :], 1.0)
    nc.vector.memset(hoT1[D:D + 1, :], 1.0)
    nc.vector.memset(v_sb[:, :, D:D + 1], 1.0)
    nc.vector.memset(v_sb[:, :, 2 * D + 1:2 * D + 2], 1.0)

    # ---- weights (small, load once up front) ----
    for i, w in enumerate([wq, wk, wv]):
        wf = stage.tile([P, KC, P], F32, tag="wstage")
        nc.sync.dma_start(out=wf, in_=w.rearrange("(k p) d -> p k d", p=P))
        nc.vector.tensor_copy(out=wbf[:, i], in_=wf)
    pjf = stage.tile([P, C], F32, tag="pjstage")
    nc.sync.dma_start(out=pjf, in_=projT)
    nc.vector.tensor_copy(out=pj0[0:D, :], in_=pjf[0:D, :])
    nc.vector.tensor_copy(out=pj1[0:D, :], in_=pjf[D:2 * D, :])
    bf1 = stage.tile([1, C], F32, tag="biasstage")
    nc.sync.dma_start(out=bf1, in_=bias16)
    nc.vector.tensor_copy(out=pj0[D:D + 1, :], in_=bf1)
    nc.vector.tensor_copy(out=pj1[D:D + 1, :], in_=bf1)

    for b in range(B):
        tb = slice(T * b, T * (b + 1))
        # ---- x^T for this batch (bf16, straight from DRAM) ----
        for k in range(KC):
            nc.sync.dma_start(out=xbf[:, k, tb], in_=xT[k * P:(k + 1) * P, tb])
        # ---- q^T, k^T for this batch ----
        for pi, dst in ((0, qTb), (1, kTb)):
            for tch in range(2 * b, 2 * (b + 1)):
                ps = psum.tile([P, 512], F32, tag="mm")
                for k in range(KC):
                    nc.tensor.matmul(
                        ps, wbf[:, pi, k, :], xbf[:, k, 512 * tch:512 * (tch + 1)],
                        start=(k == 0), stop=(k == KC - 1))
                nc.scalar.copy(out=dst[:, 512 * tch:512 * (tch + 1)], in_=ps)
        # ---- v natural for this batch ----
        for tt in range(SM * b, SM * (b + 1)):
            ps = psum.tile([P, P], F32, tag="mm")
            for k in range(KC):
                nc.tensor.matmul(
                    ps, xbf[:, k, P * tt:P * (tt + 1)], wbf[:, 2, k, :],
                    start=(k == 0), stop=(k == KC - 1))
            nc.scalar.copy(out=v_sb[:, tt, 0:D], in_=ps[:, 0:D])
            nc.scalar.copy(out=v_sb[:, tt, D + 1:2 * D + 1],
                           in_=ps[:, D:2 * D])

        # ---- attention pairs for this batch ----
        for h in range(HL):
            qs = qTb[D * h:D * (h + 1), tb]   # [64, 1024]
            ks = kTb[D * h:D * (h + 1), tb]
            pt = ppool.tile([P, SM, T], BF16, tag="pt")      # p^T, bf16

            for m in range(SM):
                v0 = P * m
                # score pieces over the valid extent [v0, T)
                a = v0
                while a < T:
                    wd = min(512, T - a)
                    ps = psum.tile([P, 512], F32, tag="mm")
                    nc.tensor.matmul(
                        ps[:, 0:wd], ks[:, v0:v0 + P], qs[:, a:a + wd],
                        start=True, stop=True)
                    nc.scalar.activation(
                        out=pt[:, m, a:a + wd], in_=ps[:, 0:wd],
                        func=Exp, scale=SCALE)
                    a += wd
                # zero p where t < s inside the diagonal block
                nc.gpsimd.affine_select(
                    pt[:, m, v0:v0 + P], pt[:, m, v0:v0 + P],
                    compare_op=mybir.AluOpType.is_ge, fill=0.0,
                    base=0, pattern=[[1, P]], channel_multiplier=-1)

            # (p @ v)^T plus ones-column -> column sums, over s-tiles
            oa = psum_oa.tile([D + 1, T], F32, tag="oa")
            for cidx in range(T // 512):
                c0, c1 = 512 * cidx, 512 * (cidx + 1)
                ms = [m for m in range(SM) if P * m < c1]
                for j, m in enumerate(ms):
                    lo = max(c0, P * m)   # nested: j==0 covers the full chunk
                    nc.tensor.matmul(
                        oa[:, lo:c1],
                        v_sb[:, SM * b + m, (D + 1) * h:(D + 1) * (h + 1)],
                        pt[:, m, lo:c1],
                        start=(j == 0), stop=(j == len(ms) - 1))

            sums = rpool.tile([1, T], F32, tag="sums")
            nc.scalar.copy(out=sums, in_=oa[D:D + 1, :])
            rec = rpool.tile([1, T], F32, tag="rec")
            nc.vector.reciprocal(out=rec, in_=sums)
            recbc = rpool.tile([P, T], F32, tag="recbc")
            nc.gpsimd.partition_broadcast(out_ap=recbc, in_ap=rec)

            hoT = hoT0 if h == 0 else hoT1
            nc.vector.tensor_mul(hoT[0:D, tb], oa[0:D, :], recbc[0:D, :])

            for m in range(SM):
                v0 = P * m
                ws = wpool.tile([P, T], BF16, tag="ws")
                eng = nc.vector if m < SM // 2 else nc.gpsimd
                eng.tensor_mul(ws[:, v0:], pt[:, m, v0:], recbc[:, v0:])
                nc.sync.dma_start(
                    out=weiT[h, b, v0:v0 + P, v0:], in_=ws[:, v0:])

        # ---- output projection for this batch's rows (K=65 adds bias) ----
        for tt in range(SM * b, SM * (b + 1)):
            for cidx in range(C // 512):
                ps = psum.tile([P, 512], F32, tag="mm")
                nc.tensor.matmul(
                    ps, hoT0[:, P * tt:P * (tt + 1)],
                    pj0[:, 512 * cidx:512 * (cidx + 1)], start=True, stop=False)
                nc.tensor.matmul(
                    ps, hoT1[:, P * tt:P * (tt + 1)],
                    pj1[:, 512 * cidx:512 * (cidx + 1)], start=False, stop=True)
                ob = opool.tile([P, 512], BF16, tag="ob")
                nc.scalar.copy(out=ob, in_=ps)
                nc.sync.dma_start(
                    out=partial[P * tt:P * (tt + 1),
                                512 * cidx:512 * (cidx + 1)],
                    in_=ob)


def build(reps=1):
    key = ("nc", reps)
    if key in _CACHED:
        return _CACHED[key]
    nc = bacc.Bacc("TRN2", target_bir_lowering=False, debug=False,
                   num_devices=N_CORES)
    xT = nc.dram_tensor("xT", [C, NT], BF16, kind="ExternalInput").ap()
    wq = nc.dram_tensor("wq", [C, HL * D], F32, kind="ExternalInput").ap()
    wk = nc.dram_tensor("wk", [C, HL * D], F32, kind="ExternalInput").ap()
    wv = nc.dram_tensor("wv", [C, HL * D], F32, kind="ExternalInput").ap()
    projT = nc.dram_tensor("projT", [HL * D, C], F32, kind="ExternalInput").ap()
    bias16 = nc.dram_tensor("bias16", [1, C], F32, kind="ExternalInput").ap()
    weiT = nc.dram_tensor("weiT", [HL, B, T, T], BF16, kind="ExternalOutput").ap()
    partial = nc.dram_tensor("partial", [NT, C], BF16, kind="ExternalOutput").ap()
    from contextlib import ExitStack
    with tile.TileContext(nc) as tc:
        for r in range(reps):
            with ExitStack() as ctx:
                _body(tc, xT, wq, wk, wv, projT, bias16, weiT, partial, ctx,
                      pfx=f"r{r}_" if reps > 1 else "")
    nc.compile()
    _CACHED[key] = nc
    return nc


def make_in_maps(x, wk, wq, wv, proj_w, proj_b):
    import ml_dtypes
    x2d = np.asarray(x, dtype=np.float32).reshape(NT, C)
    xT = np.ascontiguousarray(x2d.T).astype(ml_dtypes.bfloat16)
    projT_full = np.ascontiguousarray(np.asarray(proj_w, dtype=np.float32).T)
    bias16 = (np.asarray(proj_b, dtype=np.float32) / (2 * N_CORES)).reshape(1, C)
    wq_ = np.asarray(wq, dtype=np.float32)
    wk_ = np.asarray(wk, dtype=np.float32)
    wv_ = np.asarray(wv, dtype=np.float32)
    in_maps = []
    for c in range(N_CORES):
        hs = slice(HL * c, HL * (c + 1))
        in_maps.append({
            "xT": xT,
            "wq": np.ascontiguousarray(
                np.concatenate(list(wq_[hs]), axis=1)),
            "wk": np.ascontiguousarray(
                np.concatenate(list(wk_[hs]), axis=1)),
            "wv": np.ascontiguousarray(
                np.concatenate(list(wv_[hs]), axis=1)),
            "projT": np.ascontiguousarray(projT_full[P * c:P * (c + 1), :]),
            "bias16": bias16,
        })
    return in_maps


def assemble(results):
    wei = np.empty((H, B, T, T), dtype=np.float32)
    for c, r in enumerate(results):
        wei[HL * c:HL * (c + 1)] = np.swapaxes(
            np.asarray(r["weiT"]).astype(np.float32), -1, -2)
    out = np.zeros((NT, C), dtype=np.float32)
    for r in results:
        out += np.asarray(r["partial"]).astype(np.float32)
    return wei, out.reshape(B, T, C)


def kernel(x, wk, wq, wv, proj_w, proj_b, _run_kwargs=None):
    nc = build()
    in_maps = make_in_maps(x, wk, wq, wv, proj_w, proj_b)
    kw = dict(_run_kwargs or {})
    res = bass_utils.run_bass_kernel_spmd(
        nc, in_maps, core_ids=list(range(N_CORES)), **kw)
    _CACHED["last_results"] = res
    return assemble(res.results)


# revision 19
# speedup vs baseline: 1.5814x; 1.5814x over previous
"""Bass/Trainium2 kernel for DecodeMultiHeadAttention (16 heads, B=2, T=1024, C=1024).

Sharding: tensor-parallel over heads -- 2 heads per core x 8 cores.

Math notes (vs the jax reference):
  * The ALiBi bias is base**clip(j-i,0) which is exactly 1.0 on every causal
    (unmasked) position, and softmax is shift-invariant, so it drops out.
  * Scores are tiny (std ~0.1) so softmax needs no max-subtraction:
    wei = exp(s*scale) / sum(exp(s*scale)) over the causal extent.

Per core (2 local heads h in {0,1}, batch b in {0,1}):
  * qT,kT = [d,t]-layout projections (PE contracts embedding dim c, using a
    host-transposed xT input so all DMA is contiguous).
  * v in natural [t,d] layout with a ones-column appended, so a single PE
    matmul per pair produces both (p @ v)^T and the softmax denominator.
  * Scores are computed transposed, sT[s,t] = k_s . q_t, so the p @ v matmul
    needs no on-chip transpose. wei is written transposed ([s,t]) and the
    host swaps the last two axes while unsharding.
  * Only the causal (lower-triangle in [t,s] == upper in [s,t]) block rows are
    computed and written; the rest of the wei output buffer stays zero
    (outputs are zero-initialized).
  * Output projection: each core computes a partial product over its 128 head
    dims (+ bias/8); the host sums the 8 partials.
"""

import numpy as np

import concourse.bacc as bacc
import concourse.tile as tile
from concourse import mybir
from concourse import bass_utils

N_CORES = 8
B, T, C = 2, 1024, 1024
H, D = 16, 64          # total heads, head dim
HL = H // N_CORES      # heads per core (2)
NT = B * T             # 2048 rows total
P = 128
KC = C // P            # 8 contraction tiles over embedding dim
TT = NT // P           # 16 row tiles
SM = T // P            # 8 s-tiles per (head, batch) pair
SCALE = float(C) ** -0.5
F32 = mybir.dt.float32
BF16 = mybir.dt.bfloat16

_CACHED = {}


def _body(tc, xT, wq, wk, wv, projT, bias16, weiT, partial, ctx, pfx=""):
    nc = tc.nc
    Exp = mybir.ActivationFunctionType.Exp

    const = ctx.enter_context(tc.tile_pool(name=pfx + "const", bufs=1))
    stage = ctx.enter_context(tc.tile_pool(name=pfx + "stage", bufs=2))
    ppool = ctx.enter_context(tc.tile_pool(name=pfx + "ppool", bufs=2))
    wpool = ctx.enter_context(tc.tile_pool(name=pfx + "wpool", bufs=4))
    rpool = ctx.enter_context(tc.tile_pool(name=pfx + "rpool", bufs=2))
    opool = ctx.enter_context(tc.tile_pool(name=pfx + "opool", bufs=4))
    psum = ctx.enter_context(
        tc.tile_pool(name=pfx + "psum", bufs=2, space="PSUM"))
    psum_sc = ctx.enter_context(
        tc.tile_pool(name=pfx + "psum_sc", bufs=4, space="PSUM"))
    psum_oa = ctx.enter_context(
        tc.tile_pool(name=pfx + "psum_oa", bufs=1, space="PSUM"))

    # ---- persistent SBUF tensors ----
    xbf = const.tile([P, KC, NT], BF16)       # x^T, bf16: [c-part, c-tile, t]
    wbf = const.tile([P, 3, KC, P], BF16)     # q/k/v weights: [c-part, proj, c-tile, d]
    qTb = const.tile([P, NT], BF16)           # q^T: [2 heads x 64 d, t]
    kTb = const.tile([P, NT], BF16)
    vTb = const.tile([P, NT], BF16)
    v_sb = const.tile([P, TT, 2 * (D + 1)], BF16)  # v natural + ones cols
    ident = const.tile([P, P], BF16)          # identity for PE transpose
    nc.vector.memset(ident, 1.0)
    nc.gpsimd.affine_select(
        ident, ident, compare_op=mybir.AluOpType.is_equal, fill=0.0,
        base=0, pattern=[[-1, P]], channel_multiplier=1)
    # projT halves with a trailing bias/16 row: proj matmul K=65 adds bias
    pj0 = const.tile([D + 1, C], BF16)
    pj1 = const.tile([D + 1, C], BF16)
    # head-out^T with a trailing ones row (multiplies the bias row)
    hoT0 = const.tile([D + 1, NT], BF16)
    hoT1 = const.tile([D + 1, NT], BF16)
    nc.vector.memset(hoT0[D:D + 1, :], 1.0)
    nc.vector.memset(hoT1[D:D + 1, :], 1.0)
    nc.vector.memset(v_sb[:, :, D:D + 1], 1.0)
    nc.vector.memset(v_sb[:, :, 2 * D + 1:2 * D + 2], 1.0)

    # ---- weights (small, load once up front) ----
    for i, w in enumerate([wq, wk, wv]):
        wf = stage.tile([P, KC, P], F32, tag="wstage")
        nc.sync.dma_start(out=wf, in_=w.rearrange("(k p) d -> p k d", p=P))
        nc.vector.tensor_copy(out=wbf[:, i], in_=wf)
    pjf = stage.tile([P, C], F32, tag="pjstage")
    nc.sync.dma_start(out=pjf, in_=projT)
    nc.vector.tensor_copy(out=pj0[0:D, :], in_=pjf[0:D, :])
    nc.vector.tensor_copy(out=pj1[0:D, :], in_=pjf[D:2 * D, :])
    bf1 = stage.tile([1, C], F32, tag="biasstage")
    nc.sync.dma_start(out=bf1, in_=bias16)
    nc.vector.tensor_copy(out=pj0[D:D + 1, :], in_=bf1)
    nc.vector.tensor_copy(out=pj1[D:D + 1, :], in_=bf1)

    def emit_proj(b):
        # partial[t, o] over this core's 128 head dims; K=65 rows add bias/16
        for tt in range(SM * b, SM * (b + 1)):
            for cidx in range(C // 512):
                ps = psum.tile([P, 512], F32, tag="mm")
                nc.tensor.matmul(
                    ps, hoT0[:, P * tt:P * (tt + 1)],
                    pj0[:, 512 * cidx:512 * (cidx + 1)], start=True, stop=False)
                nc.tensor.matmul(
                    ps, hoT1[:, P * tt:P * (tt + 1)],
                    pj1[:, 512 * cidx:512 * (cidx + 1)], start=False, stop=True)
                ob = opool.tile([P, 512], BF16, tag="ob")
                if cidx % 2 == 0:
                    nc.vector.tensor_copy(out=ob, in_=ps)
                else:
                    nc.scalar.copy(out=ob, in_=ps)
                nc.sync.dma_start(
                    out=partial[P * tt:P * (tt + 1),
                                512 * cidx:512 * (cidx + 1)],
                    in_=ob)

    for b in range(B):
        tb = slice(T * b, T * (b + 1))
        # ---- x^T for this batch (bf16, straight from DRAM) ----
        for k in range(KC):
            nc.sync.dma_start(out=xbf[:, k, tb], in_=xT[k * P:(k + 1) * P, tb])
        # ---- q^T, k^T for this batch ----
        for pi, dst in ((0, qTb), (1, kTb)):
            for tch in range(2 * b, 2 * (b + 1)):
                ps = psum.tile([P, 512], F32, tag="mm")
                for k in range(KC):
                    nc.tensor.matmul(
                        ps, wbf[:, pi, k, :],
                        xbf[:, k, 512 * tch:512 * (tch + 1)],
                        start=(k == 0), stop=(k == KC - 1))
                nc.scalar.copy(out=dst[:, 512 * tch:512 * (tch + 1)], in_=ps)

        # ---- scores + exp + causal mask, h0/h1 interleaved ----
        # h0 ops use partitions 0:64, h1 64:128 -> adjacent score matmuls
        # land in different PE row groups and run concurrently.
        qsh = [qTb[D * h:D * (h + 1), tb] for h in range(HL)]
        ksh = [kTb[D * h:D * (h + 1), tb] for h in range(HL)]
        pts = [ppool.tile([P, SM, T], BF16, tag=f"pt{h}", name=f"pt{h}")
               for h in range(HL)]
        for m in range(SM):
            v0 = P * m
            a = v0
            while a < T:
                wd = min(512, T - a)
                for h in range(HL):
                    ps = psum_sc.tile([P, 512], F32, tag="sc")
                    nc.tensor.matmul(
                        ps[:, 0:wd], ksh[h][:, v0:v0 + P], qsh[h][:, a:a + wd],
                        start=True, stop=True)
                    nc.scalar.activation(
                        out=pts[h][:, m, a:a + wd], in_=ps[:, 0:wd],
                        func=Exp, scale=SCALE)
                a += wd
            for h in range(HL):
                nc.gpsimd.affine_select(
                    pts[h][:, m, v0:v0 + P], pts[h][:, m, v0:v0 + P],
                    compare_op=mybir.AluOpType.is_ge, fill=0.0,
                    base=0, pattern=[[1, P]], channel_multiplier=-1)

        # ---- v for this batch: vT matmuls + PE transposes (PE filler
        # while ACT runs the exps above) ----
        for tch in range(2 * b, 2 * (b + 1)):
            ps = psum.tile([P, 512], F32, tag="mm")
            for k in range(KC):
                nc.tensor.matmul(
                    ps, wbf[:, 2, k, :], xbf[:, k, 512 * tch:512 * (tch + 1)],
                    start=(k == 0), stop=(k == KC - 1))
            nc.vector.tensor_copy(out=vTb[:, 512 * tch:512 * (tch + 1)], in_=ps)
        for tt in range(SM * b, SM * (b + 1)):
            pv = psum.tile([P, P], BF16, tag="mm")
            nc.tensor.transpose(pv, vTb[:, P * tt:P * (tt + 1)], ident)
            nc.vector.tensor_copy(out=v_sb[:, tt, 0:D], in_=pv[:, 0:D])
            nc.vector.tensor_copy(out=v_sb[:, tt, D + 1:2 * D + 1],
                                  in_=pv[:, D:2 * D])

        if b == 1:
            emit_proj(0)   # PE filler while batch 1's exps drain

        # ---- (p @ v)^T + sums; normalize; wei out ----
        for h in range(HL):
            pt = pts[h]
            oa = psum_oa.tile([D + 1, T], F32, tag="oa")
            for cidx in range(T // 512):
                c0, c1 = 512 * cidx, 512 * (cidx + 1)
                ms = [m for m in range(SM) if P * m < c1]
                for j, m in enumerate(ms):
                    lo = max(c0, P * m)   # nested: j==0 covers the full chunk
                    nc.tensor.matmul(
                        oa[:, lo:c1],
                        v_sb[:, SM * b + m, (D + 1) * h:(D + 1) * (h + 1)],
                        pt[:, m, lo:c1],
                        start=(j == 0), stop=(j == len(ms) - 1))

            rec = rpool.tile([1, T], BF16, tag="rec")
            with nc.allow_low_precision(reason="bf16 softmax denominators"):
                nc.vector.reciprocal(out=rec, in_=oa[D:D + 1, :])
            recbc = rpool.tile([P, T], BF16, tag="recbc")
            nc.gpsimd.partition_broadcast(out_ap=recbc, in_ap=rec)

            hoT = hoT0 if h == 0 else hoT1
            nc.vector.tensor_mul(hoT[0:D, tb], oa[0:D, :], recbc[0:D, :])

            for m in range(SM):
                v0 = P * m
                ws = wpool.tile([P, T], BF16, tag="ws")
                nc.vector.tensor_mul(ws[:, v0:], pt[:, m, v0:], recbc[:, v0:])
                nc.sync.dma_start(
                    out=weiT[h, b, v0:v0 + P, v0:], in_=ws[:, v0:])

    emit_proj(1)


def build(reps=1):
    key = ("nc", reps)
    if key in _CACHED:
        return _CACHED[key]
    nc = bacc.Bacc("TRN2", target_bir_lowering=False, debug=False,
                   num_devices=N_CORES)
    xT = nc.dram_tensor("xT", [C, NT], BF16, kind="ExternalInput").ap()
    wq = nc.dram_tensor("wq", [C, HL * D], F32, kind="ExternalInput").ap()
    wk = nc.dram_tensor("wk", [C, HL * D], F32, kind="ExternalInput").ap()
    wv = nc.dram_tensor("wv", [C, HL * D], F32, kind="ExternalInput").ap()
    projT = nc.dram_tensor("projT", [HL * D, C], F32, kind="ExternalInput").ap()
    bias16 = nc.dram_tensor("bias16", [1, C], F32, kind="ExternalInput").ap()
    weiT = nc.dram_tensor("weiT", [HL, B, T, T], BF16, kind="ExternalOutput").ap()
    partial = nc.dram_tensor("partial", [NT, C], BF16, kind="ExternalOutput").ap()
    from contextlib import ExitStack
    with tile.TileContext(nc) as tc:
        for r in range(reps):
            with ExitStack() as ctx:
                _body(tc, xT, wq, wk, wv, projT, bias16, weiT, partial, ctx,
                      pfx=f"r{r}_" if reps > 1 else "")
    nc.compile()
    _CACHED[key] = nc
    return nc


def make_in_maps(x, wk, wq, wv, proj_w, proj_b):
    import ml_dtypes
    x2d = np.asarray(x, dtype=np.float32).reshape(NT, C)
    xT = np.ascontiguousarray(x2d.T).astype(ml_dtypes.bfloat16)
    projT_full = np.ascontiguousarray(np.asarray(proj_w, dtype=np.float32).T)
    bias16 = (np.asarray(proj_b, dtype=np.float32) / (2 * N_CORES)).reshape(1, C)
    wq_ = np.asarray(wq, dtype=np.float32)
    wk_ = np.asarray(wk, dtype=np.float32)
    wv_ = np.asarray(wv, dtype=np.float32)
    in_maps = []
    for c in range(N_CORES):
        hs = slice(HL * c, HL * (c + 1))
        in_maps.append({
            "xT": xT,
            "wq": np.ascontiguousarray(
                np.concatenate(list(wq_[hs]), axis=1)),
            "wk": np.ascontiguousarray(
                np.concatenate(list(wk_[hs]), axis=1)),
            "wv": np.ascontiguousarray(
                np.concatenate(list(wv_[hs]), axis=1)),
            "projT": np.ascontiguousarray(projT_full[P * c:P * (c + 1), :]),
            "bias16": bias16,
        })
    return in_maps


def assemble(results):
    wei = np.empty((H, B, T, T), dtype=np.float32)
    for c, r in enumerate(results):
        wei[HL * c:HL * (c + 1)] = np.swapaxes(
            np.asarray(r["weiT"]).astype(np.float32), -1, -2)
    out = np.zeros((NT, C), dtype=np.float32)
    for r in results:
        out += np.asarray(r["partial"]).astype(np.float32)
    return wei, out.reshape(B, T, C)


def kernel(x, wk, wq, wv, proj_w, proj_b, _run_kwargs=None):
    nc = build()
    in_maps = make_in_maps(x, wk, wq, wv, proj_w, proj_b)
    kw = dict(_run_kwargs or {})
    res = bass_utils.run_bass_kernel_spmd(
        nc, in_maps, core_ids=list(range(N_CORES)), **kw)
    _CACHED["last_results"] = res
    return assemble(res.results)


# revision 20
# speedup vs baseline: 1.6178x; 1.0231x over previous
"""Bass/Trainium2 kernel for DecodeMultiHeadAttention (16 heads, B=2, T=1024, C=1024).

Sharding: tensor-parallel over heads -- 2 heads per core x 8 cores.

Math notes (vs the jax reference):
  * The ALiBi bias is base**clip(j-i,0) which is exactly 1.0 on every causal
    (unmasked) position, and softmax is shift-invariant, so it drops out.
  * Scores are tiny (std ~0.1) so softmax needs no max-subtraction:
    wei = exp(s*scale) / sum(exp(s*scale)) over the causal extent.

Per core (2 local heads h in {0,1}, batch b in {0,1}):
  * qT,kT = [d,t]-layout projections (PE contracts embedding dim c, using a
    host-transposed xT input so all DMA is contiguous).
  * v in natural [t,d] layout with a ones-column appended, so a single PE
    matmul per pair produces both (p @ v)^T and the softmax denominator.
  * Scores are computed transposed, sT[s,t] = k_s . q_t, so the p @ v matmul
    needs no on-chip transpose. wei is written transposed ([s,t]) and the
    host swaps the last two axes while unsharding.
  * Only the causal (lower-triangle in [t,s] == upper in [s,t]) block rows are
    computed and written; the rest of the wei output buffer stays zero
    (outputs are zero-initialized).
  * Output projection: each core computes a partial product over its 128 head
    dims (+ bias/8); the host sums the 8 partials.
"""

import numpy as np

import concourse.bacc as bacc
import concourse.tile as tile
from concourse import mybir
from concourse import bass_utils

N_CORES = 8
B, T, C = 2, 1024, 1024
H, D = 16, 64          # total heads, head dim
HL = H // N_CORES      # heads per core (2)
NT = B * T             # 2048 rows total
P = 128
KC = C // P            # 8 contraction tiles over embedding dim
TT = NT // P           # 16 row tiles
SM = T // P            # 8 s-tiles per (head, batch) pair
SCALE = float(C) ** -0.5
F32 = mybir.dt.float32
BF16 = mybir.dt.bfloat16

_CACHED = {}


def _body(tc, xT, wq, wk, wv, projT, bias8, weiT, partial, ctx, pfx=""):
    nc = tc.nc
    Exp = mybir.ActivationFunctionType.Exp

    const = ctx.enter_context(tc.tile_pool(name=pfx + "const", bufs=1))
    stage = ctx.enter_context(tc.tile_pool(name=pfx + "stage", bufs=2))
    ppool = ctx.enter_context(tc.tile_pool(name=pfx + "ppool", bufs=2))
    wpool = ctx.enter_context(tc.tile_pool(name=pfx + "wpool", bufs=4))
    rpool = ctx.enter_context(tc.tile_pool(name=pfx + "rpool", bufs=2))
    opool = ctx.enter_context(tc.tile_pool(name=pfx + "opool", bufs=4))
    psum = ctx.enter_context(
        tc.tile_pool(name=pfx + "psum", bufs=2, space="PSUM"))
    psum_sc = ctx.enter_context(
        tc.tile_pool(name=pfx + "psum_sc", bufs=4, space="PSUM"))
    psum_oa = ctx.enter_context(
        tc.tile_pool(name=pfx + "psum_oa", bufs=1, space="PSUM"))

    # ---- persistent SBUF tensors ----
    xbf = const.tile([P, KC, NT], BF16)       # x^T, bf16: [c-part, c-tile, t]
    wbf = const.tile([P, 3, KC, P], BF16)     # q/k/v weights: [c-part, proj, c-tile, d]
    qTb = const.tile([P, NT], BF16)           # q^T: [2 heads x 64 d, t]
    kTb = const.tile([P, NT], BF16)
    vTb = const.tile([P, NT], BF16)
    v_sb = const.tile([P, TT, 2 * (D + 1)], BF16)  # v natural + ones cols
    ident = const.tile([P, P], BF16)          # identity for PE transpose
    nc.vector.memset(ident, 1.0)
    nc.gpsimd.affine_select(
        ident, ident, compare_op=mybir.AluOpType.is_equal, fill=0.0,
        base=0, pattern=[[-1, P]], channel_multiplier=1)
    pjall = const.tile([P, C], BF16)          # projT_c: [local head dim, o]
    hoTall = const.tile([P, NT], BF16)        # both heads' out^T: [i, t]
    biasv = const.tile([P, KC], F32)          # bias/8 as per-partition scalars
    nc.vector.memset(v_sb[:, :, D:D + 1], 1.0)
    nc.vector.memset(v_sb[:, :, 2 * D + 1:2 * D + 2], 1.0)

    # ---- weights (small, load once up front) ----
    for i, w in enumerate([wq, wk, wv]):
        wf = stage.tile([P, KC, P], F32, tag="wstage")
        nc.sync.dma_start(out=wf, in_=w.rearrange("(k p) d -> p k d", p=P))
        nc.vector.tensor_copy(out=wbf[:, i], in_=wf)
    pjf = stage.tile([P, C], F32, tag="pjstage")
    nc.sync.dma_start(out=pjf, in_=projT)
    nc.vector.tensor_copy(out=pjall, in_=pjf)
    nc.sync.dma_start(
        out=biasv, in_=bias8.rearrange("one (ot p) -> (one p) ot", p=P))

    def emit_proj(b):
        # partial^T[o, t] = projT_c^T @ hoT over this core's 128 head dims;
        # bias/8 rides along as the ACT copy's per-partition bias.
        Ident = mybir.ActivationFunctionType.Identity
        for ot in range(C // P):
            ob = opool.tile([P, 2, 512], BF16, tag="ob")
            for ci in range(2):
                tch = 2 * b + ci
                ps = psum.tile([P, 512], F32, tag="mm")
                nc.tensor.matmul(
                    ps, pjall[:, P * ot:P * (ot + 1)],
                    hoTall[:, 512 * tch:512 * (tch + 1)],
                    start=True, stop=True)
                nc.scalar.activation(
                    out=ob[:, ci, :], in_=ps, func=Ident,
                    bias=biasv[:, ot:ot + 1], scale=1.0)
            nc.sync.dma_start(
                out=partial[P * ot:P * (ot + 1), T * b:T * (b + 1)],
                in_=ob.rearrange("p a c -> p (a c)"))

    for b in range(B):
        tb = slice(T * b, T * (b + 1))
        # ---- x^T for this batch (bf16, straight from DRAM) ----
        for k in range(KC):
            nc.sync.dma_start(out=xbf[:, k, tb], in_=xT[k * P:(k + 1) * P, tb])
        # ---- q^T, k^T for this batch ----
        for pi, dst in ((0, qTb), (1, kTb)):
            for tch in range(2 * b, 2 * (b + 1)):
                ps = psum.tile([P, 512], F32, tag="mm")
                for k in range(KC):
                    nc.tensor.matmul(
                        ps, wbf[:, pi, k, :],
                        xbf[:, k, 512 * tch:512 * (tch + 1)],
                        start=(k == 0), stop=(k == KC - 1))
                nc.scalar.copy(out=dst[:, 512 * tch:512 * (tch + 1)], in_=ps)

        # ---- scores + exp + causal mask, h0/h1 interleaved ----
        # h0 ops use partitions 0:64, h1 64:128 -> adjacent score matmuls
        # land in different PE row groups and run concurrently.
        qsh = [qTb[D * h:D * (h + 1), tb] for h in range(HL)]
        ksh = [kTb[D * h:D * (h + 1), tb] for h in range(HL)]
        pts = [ppool.tile([P, SM, T], BF16, tag=f"pt{h}", name=f"pt{h}")
               for h in range(HL)]
        for m in range(SM):
            v0 = P * m
            a = v0
            while a < T:
                wd = min(512, T - a)
                for h in range(HL):
                    ps = psum_sc.tile([P, 512], F32, tag="sc")
                    nc.tensor.matmul(
                        ps[:, 0:wd], ksh[h][:, v0:v0 + P], qsh[h][:, a:a + wd],
                        start=True, stop=True)
                    nc.scalar.activation(
                        out=pts[h][:, m, a:a + wd], in_=ps[:, 0:wd],
                        func=Exp, scale=SCALE)
                a += wd
            for h in range(HL):
                nc.gpsimd.affine_select(
                    pts[h][:, m, v0:v0 + P], pts[h][:, m, v0:v0 + P],
                    compare_op=mybir.AluOpType.is_ge, fill=0.0,
                    base=0, pattern=[[1, P]], channel_multiplier=-1)

        # ---- v for this batch: vT matmuls + PE transposes (PE filler
        # while ACT runs the exps above) ----
        for tch in range(2 * b, 2 * (b + 1)):
            ps = psum.tile([P, 512], F32, tag="mm")
            for k in range(KC):
                nc.tensor.matmul(
                    ps, wbf[:, 2, k, :], xbf[:, k, 512 * tch:512 * (tch + 1)],
                    start=(k == 0), stop=(k == KC - 1))
            nc.vector.tensor_copy(out=vTb[:, 512 * tch:512 * (tch + 1)], in_=ps)
        for tt in range(SM * b, SM * (b + 1)):
            pv = psum.tile([P, P], BF16, tag="mm")
            nc.tensor.transpose(pv, vTb[:, P * tt:P * (tt + 1)], ident)
            nc.vector.tensor_copy(out=v_sb[:, tt, 0:D], in_=pv[:, 0:D])
            nc.vector.tensor_copy(out=v_sb[:, tt, D + 1:2 * D + 1],
                                  in_=pv[:, D:2 * D])

        if b == 1:
            emit_proj(0)   # PE filler while batch 1's exps drain

        # ---- (p @ v)^T + sums; normalize; wei out ----
        for h in range(HL):
            pt = pts[h]
            oa = psum_oa.tile([D + 1, T], F32, tag="oa")
            for cidx in range(T // 512):
                c0, c1 = 512 * cidx, 512 * (cidx + 1)
                ms = [m for m in range(SM) if P * m < c1]
                for j, m in enumerate(ms):
                    lo = max(c0, P * m)   # nested: j==0 covers the full chunk
                    nc.tensor.matmul(
                        oa[:, lo:c1],
                        v_sb[:, SM * b + m, (D + 1) * h:(D + 1) * (h + 1)],
                        pt[:, m, lo:c1],
                        start=(j == 0), stop=(j == len(ms) - 1))

            rec = rpool.tile([1, T], BF16, tag="rec")
            with nc.allow_low_precision(reason="bf16 softmax denominators"):
                nc.vector.reciprocal(out=rec, in_=oa[D:D + 1, :])
            recbc = rpool.tile([P, T], BF16, tag="recbc")
            nc.gpsimd.partition_broadcast(out_ap=recbc, in_ap=rec)

            if h == 0:
                nc.vector.tensor_mul(hoTall[0:D, tb], oa[0:D, :],
                                     recbc[0:D, :])
            else:
                h1t = rpool.tile([D, T], BF16, tag="h1t")
                nc.vector.tensor_mul(h1t, oa[0:D, :], recbc[0:D, :])
                nc.sync.dma_start(out=hoTall[D:2 * D, tb], in_=h1t)

            for m in range(SM):
                v0 = P * m
                ws = wpool.tile([P, T], BF16, tag="ws")
                nc.vector.tensor_mul(ws[:, v0:], pt[:, m, v0:], recbc[:, v0:])
                nc.sync.dma_start(
                    out=weiT[h, b, v0:v0 + P, v0:], in_=ws[:, v0:])

    emit_proj(1)


def build(reps=1):
    key = ("nc", reps)
    if key in _CACHED:
        return _CACHED[key]
    nc = bacc.Bacc("TRN2", target_bir_lowering=False, debug=False,
                   num_devices=N_CORES)
    xT = nc.dram_tensor("xT", [C, NT], BF16, kind="ExternalInput").ap()
    wq = nc.dram_tensor("wq", [C, HL * D], F32, kind="ExternalInput").ap()
    wk = nc.dram_tensor("wk", [C, HL * D], F32, kind="ExternalInput").ap()
    wv = nc.dram_tensor("wv", [C, HL * D], F32, kind="ExternalInput").ap()
    projT = nc.dram_tensor("projT", [HL * D, C], F32, kind="ExternalInput").ap()
    bias8 = nc.dram_tensor("bias8", [1, C], F32, kind="ExternalInput").ap()
    weiT = nc.dram_tensor("weiT", [HL, B, T, T], BF16, kind="ExternalOutput").ap()
    partial = nc.dram_tensor("partial", [C, NT], BF16, kind="ExternalOutput").ap()
    from contextlib import ExitStack
    with tile.TileContext(nc) as tc:
        for r in range(reps):
            with ExitStack() as ctx:
                _body(tc, xT, wq, wk, wv, projT, bias8, weiT, partial, ctx,
                      pfx=f"r{r}_" if reps > 1 else "")
    nc.compile()
    _CACHED[key] = nc
    return nc


def make_in_maps(x, wk, wq, wv, proj_w, proj_b):
    import ml_dtypes
    x2d = np.asarray(x, dtype=np.float32).reshape(NT, C)
    xT = np.ascontiguousarray(x2d.T).astype(ml_dtypes.bfloat16)
    projT_full = np.ascontiguousarray(np.asarray(proj_w, dtype=np.float32).T)
    bias8 = (np.asarray(proj_b, dtype=np.float32) / N_CORES).reshape(1, C)
    wq_ = np.asarray(wq, dtype=np.float32)
    wk_ = np.asarray(wk, dtype=np.float32)
    wv_ = np.asarray(wv, dtype=np.float32)
    in_maps = []
    for c in range(N_CORES):
        hs = slice(HL * c, HL * (c + 1))
        in_maps.append({
            "xT": xT,
            "wq": np.ascontiguousarray(
                np.concatenate(list(wq_[hs]), axis=1)),
            "wk": np.ascontiguousarray(
                np.concatenate(list(wk_[hs]), axis=1)),
            "wv": np.ascontiguousarray(
                np.concatenate(list(wv_[hs]), axis=1)),
            "projT": np.ascontiguousarray(projT_full[P * c:P * (c + 1), :]),
            "bias8": bias8,
        })
    return in_maps


def assemble(results):
    wei = np.empty((H, B, T, T), dtype=np.float32)
    for c, r in enumerate(results):
        wei[HL * c:HL * (c + 1)] = np.swapaxes(
            np.asarray(r["weiT"]).astype(np.float32), -1, -2)
    out = np.zeros((NT, C), dtype=np.float32)
    for r in results:
        out += np.asarray(r["partial"]).astype(np.float32).T
    return wei, out.reshape(B, T, C)


def kernel(x, wk, wq, wv, proj_w, proj_b, _run_kwargs=None):
    nc = build()
    in_maps = make_in_maps(x, wk, wq, wv, proj_w, proj_b)
    kw = dict(_run_kwargs or {})
    res = bass_utils.run_bass_kernel_spmd(
        nc, in_maps, core_ids=list(range(N_CORES)), **kw)
    _CACHED["last_results"] = res
    return assemble(res.results)
